# revision 17
# baseline (speedup 1.0000x reference)
"""Top-2 MoE (8 experts, d_model=1024, d_ff=4096) on 8 Trainium2 NeuronCores.

Strategy: expert parallelism. The tiny router (softmax top-2 over 8 experts)
runs on the host as part of input sharding — the host must decide which
tokens go to which core anyway. Each core holds one expert's weights (bf16,
SBUF-resident) and receives exactly the tokens routed to it (padded to a
common capacity), laid out transposed so tokens live on the matmul free dim
and D/F on partitions. On-device per core:

    h^T = gelu(w1-tiles.T @ x^T)     (PE + ACT, bf16 in / f32 psum)
    y^T = w2-tiles.T @ h^T           (PE, f32 out)

The host applies the top-2 gate weights during the scatter-add combine
(gating is a linear per-token scale, applied in f32 on the host).

Weights are shipped pre-tiled in chunk-major layouts so each DMA chunk is a
separate SBUF tile (Tile tracks deps per tile -> matmuls start as soon as
the first chunk + first token tile land, not after all 16 MB of weights).
"""

import numpy as np
import ml_dtypes

D = 1024
F = 4096
E = 8
TOP_K = 2
P = 128
NT_MAX = 512   # tokens per matmul (one f32 PSUM bank)
FC = 256       # w1 f-column chunk size (per DMA chunk / SBUF tile)
KD = D // P    # 8 contraction tiles for mm1
MF = F // P    # 32 row-tiles of F (mm1 out / mm2 contraction)
MD = D // P    # 8 row-tiles of D (mm2 out)
NW1C = F // FC     # 16 w1 chunks
NW2C = 4           # w2 chunks (along kf)
KFC = MF // NW2C   # 8 kf per w2 chunk

_compiled_cache = {}


def _token_tiles(cap):
    tiles = [NT_MAX] * (cap // NT_MAX)
    if cap % NT_MAX:
        tiles.append(cap % NT_MAX)
    return tiles


def _build_bass(cap):
    import concourse.mybir as mybir
    import concourse.tile as tile
    from concourse import bacc

    bf16 = mybir.dt.bfloat16
    f32 = mybir.dt.float32

    nc = bacc.Bacc("TRN2", target_bir_lowering=False, debug=False, num_devices=E)

    # host-pretiled layouts (see _run for construction):
    #   xt: [D, cap]            row kd*128+pi, col = token
    #   w1: [NW1C, 128, KD, FC] chunk-major; (c,pi,kd,f) = w1[kd*128+pi, c*FC+f]
    #   w2: [128, MF, D]        (pi,kf,d) = w2[kf*128+pi, d]
    #   yt: [D, cap]
    xt_d = nc.dram_tensor("xt", [D * cap], bf16, kind="ExternalInput")
    w1_d = nc.dram_tensor("w1", [NW1C, P, KD, FC], bf16, kind="ExternalInput")
    w2_d = nc.dram_tensor("w2", [P, MF, D], bf16, kind="ExternalInput")
    yt_d = nc.dram_tensor("yt", [D, cap], bf16, kind="ExternalOutput")

    yt_t = yt_d.ap().rearrange("(po pi) c -> pi po c", pi=P)  # [128, MD, cap]

    tiles = _token_tiles(cap)

    with tile.TileContext(nc) as tc:
        with (
            tc.tile_pool(name="wpool", bufs=1) as wpool,
            tc.tile_pool(name="xpool", bufs=2) as xpool,
            tc.tile_pool(name="hpool", bufs=1) as hpool,
            tc.tile_pool(name="ypool", bufs=2) as ypool,
            tc.tile_pool(name="ps1", bufs=3, space="PSUM") as ps1,
            tc.tile_pool(name="ps2", bufs=3, space="PSUM") as ps2,
            tc.tile_pool(name="psw", bufs=1, space="PSUM") as psw,
        ):
            # Warm the PE HAM clock gate during the startup DMA window with
            # dummy matmuls on a zeroed tile (PE is otherwise idle ~15us and
            # would start the real stream at 1.2 GHz).
            wz = wpool.tile([P, P], bf16, tag="warm")
            nc.any.memzero(wz[:])
            pw = psw.tile([P, P], f32, tag="psw")
            for _ in range(30):
                nc.tensor.matmul(pw[:], wz[:], wz[:], start=True, stop=True)

            # token tile 0 on the SP DMA ring; weights in parallel on the
            # ACT ring. First w1 chunk + x0 gate the first real matmul.
            w1c = [wpool.tile([P, KD, FC], bf16, tag="w1c0", name="w1c0")]
            nc.sync.dma_start(w1c[0][:], w1_d.ap()[0])

            def x_src(col, nt):  # [128, KD, nt], 8KB contiguous/partition
                return xt_d.ap()[D * col : D * (col + nt)].rearrange(
                    "(pi kd j) -> pi kd j", pi=P, kd=KD
                )

            xsb = []
            x0 = xpool.tile([P, KD, NT_MAX], bf16, tag="x")
            nc.sync.dma_start(x0[:, :, : tiles[0]], x_src(0, tiles[0]))
            xsb.append(x0)

            for c in range(1, NW1C):
                w = wpool.tile([P, KD, FC], bf16, tag=f"w1c{c}", name=f"w1c{c}")
                nc.sync.dma_start(w[:], w1_d.ap()[c])
                w1c.append(w)
            w2c = []
            for j in range(NW2C):
                w = wpool.tile([P, KFC, D], bf16, tag=f"w2c{j}")
                nc.sync.dma_start(w[:], w2_d.ap()[:, j * KFC : (j + 1) * KFC, :])
                w2c.append(w)

            def w1_tile(kd, mf):  # lhsT [128(kd-part), 128 f-cols]
                c, q = divmod(mf, FC // P)
                return w1c[c][:, kd, q * P : (q + 1) * P]

            def w2_tile(kf, md):  # lhsT [128(kf-part), 128 d-cols]
                j, r = divmod(kf, KFC)
                return w2c[j][:, r, md * P : (md + 1) * P]

            col = 0
            for ct, nt in enumerate(tiles):
                if ct + 1 < len(tiles):  # prefetch next token tile
                    nxt = tiles[ct + 1]
                    xn = xpool.tile([P, KD, NT_MAX], bf16, tag="x")
                    nc.sync.dma_start(xn[:, :, :nxt], x_src(col + nt, nxt))
                    xsb.append(xn)

                # h split into 4 sub-tiles (8 kf each) so mm2 can start as
                # soon as the first 8 gelu tiles land, not after all 32.
                hsb = [
                    hpool.tile(
                        [P, MF // 4, NT_MAX], bf16, tag=f"h{i}", name=f"h{i}_{ct}"
                    )
                    for i in range(4)
                ]
                for mf in range(MF):
                    pt = ps1.tile([P, NT_MAX], f32, tag="ps1")
                    for kd in range(KD):
                        nc.tensor.matmul(
                            pt[:, :nt],
                            w1_tile(kd, mf),
                            xsb[ct][:, kd, :nt],
                            start=(kd == 0),
                            stop=(kd == KD - 1),
                        )
                    nc.scalar.activation(
                        hsb[mf // 8][:, mf % 8, :nt],
                        pt[:, :nt],
                        mybir.ActivationFunctionType.Gelu,
                    )

                ysb = ypool.tile([P, MD, NT_MAX], bf16, tag="y")
                for md in range(MD):
                    pt2 = ps2.tile([P, NT_MAX], f32, tag="ps2")
                    for kf in range(MF):
                        nc.tensor.matmul(
                            pt2[:, :nt],
                            w2_tile(kf, md),
                            hsb[kf // 8][:, kf % 8, :nt],
                            start=(kf == 0),
                            stop=(kf == MF - 1),
                        )
                    nc.vector.tensor_copy(ysb[:, md, :nt], pt2[:, :nt])
                    nc.sync.dma_start(
                        yt_t[:, md, col : col + nt], ysb[:, md, :nt]
                    )
                col += nt

    nc.compile()
    return nc


def _route(xf, w_router):
    """Host router: replicates reference softmax/top-2 math in f32 numpy.

    Selection only depends on the logit ordering (softmax is monotonic);
    gates = softmax over the two selected logits.
    """
    logits = xf @ w_router.T.astype(np.float32)  # [T, E]
    top2 = np.argpartition(-logits, 1, axis=1)[:, :2]  # unordered top-2 set
    sel = np.take_along_axis(logits, top2, axis=1)
    sel = sel - sel.max(axis=1, keepdims=True)
    ex = np.exp(sel)
    gates = ex / ex.sum(axis=1, keepdims=True)  # [T, 2]
    return top2, gates


def _run(x, w_router, w1, w2, trace=False):
    from concourse.bass_utils import run_bass_kernel_spmd

    B, S, _ = x.shape
    xf = np.ascontiguousarray(x.reshape(-1, D).astype(np.float32))
    T = xf.shape[0]

    top2, gates = _route(xf, w_router)

    rows_e, gate_e = [], []
    for e in range(E):
        rows, slot = np.nonzero(top2 == e)
        rows_e.append(rows)
        gate_e.append(gates[rows, slot])

    max_load = max(len(r) for r in rows_e)
    cap = max_load

    if cap not in _compiled_cache:
        _compiled_cache[cap] = _build_bass(cap)
    nc = _compiled_cache[cap]

    bf16 = ml_dtypes.bfloat16
    in_maps = []
    for e in range(E):
        n = len(rows_e[e])
        xe = np.zeros((cap, D), dtype=np.float32)
        xe[:n] = xf[rows_e[e]]
        xflat = np.empty(D * cap, dtype=bf16)
        off = 0
        for nt in _token_tiles(cap):
            blk = xe[off : off + nt].reshape(nt, KD, P).transpose(2, 1, 0)
            xflat[D * off : D * (off + nt)] = blk.astype(bf16).ravel()
            off += nt
        # w1[e]: [D, F] -> chunk-major [NW1C, 128, KD, FC]
        w1t = np.ascontiguousarray(
            w1[e].reshape(KD, P, NW1C, FC).transpose(2, 1, 0, 3)
        ).astype(bf16)
        # w2[e]: [F, D] -> [128, MF, D]
        w2t = np.ascontiguousarray(
            w2[e].reshape(MF, P, D).transpose(1, 0, 2)
        ).astype(bf16)
        in_maps.append(
            {
                "xt": xflat,  # tile-chunk-major, see x_src
                "w1": w1t,
                "w2": w2t,
            }
        )

    res = run_bass_kernel_spmd(
        nc, in_maps, core_ids=list(range(E)), trace=trace
    )

    out = np.zeros((T, D), dtype=np.float32)
    for e in range(E):
        n = len(rows_e[e])
        if n == 0:
            continue
        yt = res.results[e]["yt"].astype(np.float32)  # [D, cap] bf16 -> f32
        out[rows_e[e]] += gate_e[e][:, None] * yt[:, :n].T
    return out.reshape(B, S, D), res


def kernel(x, w_router, w1, w2):
    out, _ = _run(x, w_router, w1, w2, trace=False)
    return out
